# revision 46
# baseline (speedup 1.0000x reference)
"""DiGCNNet forward on 8 Trainium2 NeuronCores, data-parallel over batch.

Math (per batch b):
  adj = mean_t graph_sigs[b]                  # [30, 30]
  xw  = real[b] @ W                           # [30, 256]
  agg = adj^T @ xw + conv_bias                # [30, 256]
  h   = relu(agg)
  ns  = h @ pool_w + pool_b                   # [30]
  lg  = ns @ head_w^T + head_b                # [7]
  out = softmax(lg)

Final design (64 batches/core, 16 groups of 4, 4 quarters of 16):
  - gs shipped fp8e4m3 (quarter DMA traffic); T-reduce as fp8 DoubleRow
    matmuls (K=256: 4 batches per mm) accumulating a quarter into a PSUM
    tile [16, 900]; rel err ~8e-3 (tolerance 2e-2).
  - real shipped fp16, padded to 32 cols/batch, as 4 independent tiles so
    early groups don't wait on the whole tensor.
  - adjacency: one [16,900] PSUM->SBUF copy per quarter (ACT/DVE alternate)
    into padded [16, 960] staging rows (pad slots pre-set to 1.0), then ONE
    gpsimd reshape DMA per group -> ablk[128, g, 32] (the 1.0 pads become
    the K=31 conv_bias ones-rows).
  - agg: per-batch fp16 matmuls on 32-aligned PE quadrants; conv_bias rows
    injected into xwb by the PSUM->SBUF tensor_add with cbfull; the xw of
    group g+1 is emitted before the agg of group g (software pipelining).
  - pool: W pre-scaled by |pool_w|, columns permuted positives-first; ACT
    relu+accum gives nsP, DVE max+accum gives nsN; head matmuls (batched at
    the end) compute nsP@hw - nsN@hw via rhs [128, 2] -> out [28, 2].
  - head bias folded into the exp() bias AP; softmax tail on [28, 16] once.
"""

from contextlib import ExitStack

import numpy as np
import ml_dtypes

import concourse.bacc as bacc
import concourse.bass as bass
import concourse.tile as tile
from concourse import mybir
from concourse.bass_utils import run_bass_kernel_spmd

F32 = mybir.dt.float32
F16 = mybir.dt.float16
F8 = mybir.dt.float8e4
F16NP = np.float16
F8NP = ml_dtypes.float8_e4m3

B, T, N = 512, 64, 30
F_IN, D, C = 512, 256, 7
NCORES = 8
BL = B // NCORES        # 64 batches per core
NN = N * N              # 900
NG = 16                 # groups of 4 batches
NP = 32                 # padded per-batch stride (partitions / realt cols)


def _build_nc(k):
    """k = number of (permuted-first) non-negative pool_w columns."""
    assert 1 <= k <= D - 1
    nc = bacc.Bacc(None, target_bir_lowering=False)

    # gs pre-tiled fp8: [8 loads, 128=(b&1,t) part, (h, u-pair, i*30+j)]
    gs = nc.dram_tensor("gs", (8, 128, 2, 2, NN), F8, kind="ExternalInput")
    rtp = nc.dram_tensor("rtp", (8, F_IN, 256), F16, kind="ExternalInput")
    onesd = nc.dram_tensor("onesd", (128, 128), F8, kind="ExternalInput")
    wo = nc.dram_tensor("wo", (128, 4 * D), F16, kind="ExternalInput")
    cbhw = nc.dram_tensor("cbhw", (128, D + 4 * C), F32, kind="ExternalInput")
    h5 = nc.dram_tensor("h5", (4 * C, 5), F32, kind="ExternalInput")
    b7t = nc.dram_tensor("b7t", (4, 4 * C), F32, kind="ExternalInput")
    out = nc.dram_tensor("out", (BL, C), F32, kind="ExternalOutput")

    with tile.TileContext(nc) as tc, ExitStack() as ctx:
        consts = ctx.enter_context(tc.tile_pool(name="consts", bufs=1))
        gt_pool = ctx.enter_context(tc.tile_pool(name="gt", bufs=8))
        xwb_pool = ctx.enter_context(tc.tile_pool(name="xwb", bufs=3))
        h_pool = ctx.enter_context(tc.tile_pool(name="h", bufs=2))
        tail_pool = ctx.enter_context(tc.tile_pool(name="tail", bufs=1))
        adjp_pool = ctx.enter_context(
            tc.tile_pool(name="adjp", bufs=1, space=bass.MemorySpace.PSUM)
        )
        xwp_pool = ctx.enter_context(
            tc.tile_pool(name="xwp", bufs=2, space=bass.MemorySpace.PSUM)
        )
        aggp_pool = ctx.enter_context(
            tc.tile_pool(name="aggp", bufs=2, space=bass.MemorySpace.PSUM)
        )
        logp_pool = ctx.enter_context(
            tc.tile_pool(name="logp", bufs=1, space=bass.MemorySpace.PSUM)
        )

        def load_const(dram, shape, dtype):
            t = consts.tile(shape, dtype, tag=dram.name)
            nc.scalar.dma_start(t[:], dram[:])
            return t

        # tiny selectors first (the first T-reduce blocks on them)
        onesd_sb = load_const(onesd, [128, 128], F8)
        wo_sb = load_const(wo, [128, 4 * D], F16)
        cbhw_sb = load_const(cbhw, [128, D + 4 * C], F32)
        h5_sb = load_const(h5, [4 * C, 5], F32)
        b7t_sb = load_const(b7t, [4, 4 * C], F32)
        wt_sb = wo_sb[:, 0 : 4 * D].rearrange("p (c d) -> p c d", c=4)
        ones_dr = onesd_sb[:].rearrange("p (u h m) -> p u h m", u=4, h=2)
        cb_sb = cbhw_sb[:, 0:D]
        hw_sb = cbhw_sb[:, D : D + 4 * C]
        hbb_sb = h5_sb[:, 0:1]
        b7_sb = h5_sb[:, 1:5]

        rtq = []
        for rc in range(8):
            t = consts.tile([128, 4, 256], F16, tag=f"rtq{rc}")
            nc.scalar.dma_start(t[:], rtp[rc].rearrange("(c p) m -> p c m", p=128))
            rtq.append(t)

        # block-adjacency store: [128, 16 groups, 32]; all-ones memset gives
        # finite pad cols (30,31) for the M=32 agg matmuls
        ablk = consts.tile([128, NG, NP], F16, tag="ablk")
        nc.vector.memset(ablk[:], 1.0)

        # adjacency staging rows, padded to 32 elem-slots per i so the
        # per-group reshape DMA is a clean 2D->2D partition split; the pad
        # slots (1.0) become the K=31 conv_bias ones-rows in ablk
        adjs_a = consts.tile([16, NP * N], F16, tag="adjs_a")
        adjs_b = consts.tile([16, NP * N], F16, tag="adjs_b")
        nc.vector.memset(adjs_a[:, NN : NP * N], 1.0)
        nc.vector.memset(adjs_b[:, NN : NP * N], 1.0)

        ns_all = consts.tile([128, NG, 2], F32, tag="ns_all")
        logp_t = logp_pool.tile([4 * C, 2 * NG], F32, tag="logits")

        gtiles = []
        for v in range(8):
            gt = gt_pool.tile([128, 2, 2, NN], F8, tag="gt")
            nc.sync.dma_start(gt[:], gs[v])
            gtiles.append(gt)

        def emit_tred(q, u2, adjp_t):
            gt = gtiles[2 * q + u2 // 2]
            uu = u2 % 2
            for c0, c1 in ((0, 512), (512, NN)):
                nc.tensor.matmul(
                    adjp_t[:, c0:c1], ones_dr[:, u2, :, :], gt[:, :, uu, c0:c1],
                    start=(u2 == 0), stop=(u2 == 3),
                    perf_mode=mybir.MatmulPerfMode.DoubleRow,
                )

        def emit_adj_finish(q, adjp_t):
            adjs_t = adjs_a if q % 2 == 0 else adjs_b
            if q % 2 == 0:
                nc.scalar.copy(adjs_t[:, 0:NN], adjp_t[:])
            else:
                nc.vector.tensor_copy(adjs_t[:, 0:NN], adjp_t[:])
            for g2 in range(4):
                g = 4 * q + g2
                nc.gpsimd.dma_start(
                    ablk[:, g, 0:N], adjs_t[4 * g2 : 4 * g2 + 4, :]
                )

        xwbs = {}

        def emit_xw(g):
            xwp_t = xwp_pool.tile([128, D], F32, tag="xwp")
            for c4 in range(4):
                nc.tensor.matmul(
                    xwp_t[:],
                    rtq[g // 2][:, c4, 128 * (g % 2) : 128 * (g % 2 + 1)],
                    wt_sb[:, c4, :],
                    start=(c4 == 0), stop=(c4 == 3),
                )
            xwb_t = xwb_pool.tile([128, D], F16, tag="xwb")
            nc.vector.tensor_add(xwb_t[:], xwp_t[:], cb_sb[:])
            xwbs[g] = xwb_t

        def emit_rest(g):
            xwb_t = xwbs.pop(g)
            aggp_t = aggp_pool.tile([128, D], F32, tag="aggp")
            for b in range(4):
                p0 = NP * b
                nc.tensor.matmul(
                    aggp_t[p0 : p0 + NP, :],
                    ablk[p0 : p0 + 31, g, :],
                    xwb_t[p0 : p0 + 31, :],
                    start=True, stop=True, tile_position=(p0, p0),
                )
            h_t = h_pool.tile([128, D], F32, tag="h")
            nc.scalar.activation(
                h_t[:, 0:k], aggp_t[:, 0:k],
                mybir.ActivationFunctionType.Relu, accum_out=ns_all[:, g, 0:1],
            )
            nc.vector.tensor_scalar(
                h_t[:, k:D], aggp_t[:, k:D], 0.0, 0.0,
                mybir.AluOpType.max, mybir.AluOpType.add,
                accum_out=ns_all[:, g, 1:2],
            )

        # ---- pipelined emission: 4 quarters of 16 batches; the xw of
        # group g+1 is emitted before the agg of group g so the PE keeps
        # streaming through the xwb (DVE) latency
        for q in range(4):
            adjp_t = adjp_pool.tile([16, NN], F32, tag="adjp")
            for u2 in range(4):
                emit_tred(q, u2, adjp_t)
                if q >= 1:
                    g = 4 * (q - 1) + u2
                    emit_xw(g)
                    if g >= 1:
                        emit_rest(g - 1)
            emit_adj_finish(q, adjp_t)
        for g2 in range(4):
            emit_xw(12 + g2)
            emit_rest(11 + g2)
        emit_rest(15)

        # ---- head matmuls batched: lg[28, 2] per group ----
        for g in range(NG):
            nc.tensor.matmul(
                logp_t[:, 2 * g : 2 * g + 2], hw_sb, ns_all[:, g, :],
                start=True, stop=True,
            )

        # ---- softmax tail over the 7-class blocks ----
        lgs_t = tail_pool.tile([4 * C, 2 * NG], F32, tag="lgs")
        nc.vector.tensor_copy(lgs_t[:], logp_t[:])
        lgd_t = tail_pool.tile([4 * C, NG], F32, tag="lgd")
        nc.vector.tensor_sub(
            lgd_t[:],
            lgs_t[:].rearrange("p (g two) -> p two g", two=2)[:, 0, :],
            lgs_t[:].rearrange("p (g two) -> p two g", two=2)[:, 1, :],
        )
        e_t = tail_pool.tile([4 * C, NG], F32, tag="e")
        nc.scalar.activation(
            e_t[:], lgd_t[:], mybir.ActivationFunctionType.Exp, bias=hbb_sb,
        )
        # tail matmuls reuse sub-regions of the (already consumed) logits bank
        sum_p = logp_t[0:4, 0:NG]
        nc.tensor.matmul(sum_p, b7_sb, e_t[:], start=True, stop=True)
        ssb_t = tail_pool.tile([4, NG], F32, tag="ssb")
        nc.vector.tensor_copy(ssb_t[:], sum_p)
        bc_p = logp_t[:, NG : 2 * NG]
        nc.tensor.matmul(bc_p, b7t_sb[:], ssb_t[:], start=True, stop=True)
        rs_t = tail_pool.tile([4 * C, NG], F32, tag="rs")
        nc.vector.reciprocal(rs_t[:], bc_p)
        res_t = tail_pool.tile([4 * C, NG], F32, tag="res")
        nc.vector.tensor_mul(res_t[:], e_t[:], rs_t[:])
        nc.sync.dma_start(out.rearrange("(g bi) c -> (bi c) g", bi=4), res_t[:])

    nc.compile()
    return nc


_NC_CACHE = {}


def _get_nc(k):
    if k not in _NC_CACHE:
        _NC_CACHE[k] = _build_nc(k)
    return _NC_CACHE[k]


def _f32(x):
    return np.asarray(x, dtype=np.float32)


def _prepare(real, graph_sigs, W, conv_bias, pool_w, pool_b, head_w, head_b):
    real = _f32(real)
    graph_sigs = _f32(graph_sigs)
    W = _f32(W)
    conv_bias = _f32(conv_bias)
    pool_w = _f32(pool_w)
    head_w = _f32(head_w)
    head_b = _f32(head_b)

    # permute feature columns: non-negative pool_w first; fold |pool_w| into W
    nonneg = pool_w >= 0
    perm = np.argsort(~nonneg, kind="stable")
    k = int(nonneg.sum())
    apw = np.abs(pool_w)[perm]
    Wp = np.ascontiguousarray((W[:, perm] * apw[None, :]).astype(F16NP))
    cbp = (conv_bias[perm] * apw).astype(np.float32)

    wt = np.ascontiguousarray(Wp.reshape(4, 128, D).transpose(1, 0, 2))

    # DoubleRow selectors: m = 4*u + 2*h + (p//64), u = within-load half
    ones_dr = np.zeros((2, 64, 4, 2, 16), dtype=F8NP)
    for c in range(2):
        for u in range(4):
            for h in range(2):
                ones_dr[c, :, u, h, 4 * u + 2 * h + c] = F8NP(1.0 / T)
    ones_dr = ones_dr.reshape(128, 128)

    cbfull = np.zeros((128, D), dtype=np.float32)
    for b in range(4):
        cbfull[NP * b + N, :] = cbp

    hwblk = np.zeros((128, 4 * C), dtype=np.float32)
    for b in range(4):
        hwblk[NP * b : NP * b + N, C * b : C * (b + 1)] = head_w.T
    hb_eff = head_b + np.float32(np.asarray(pool_b)) * head_w.sum(axis=1)
    hbb = np.tile(hb_eff, 4).reshape(4 * C, 1).astype(np.float32)

    b7 = np.zeros((4 * C, 4), dtype=np.float32)
    for b in range(4):
        b7[C * b : C * (b + 1), b] = 1.0
    b7t = np.ascontiguousarray(b7.T)

    cbhw = np.concatenate([cbfull, hwblk], axis=1)
    h5 = np.concatenate([hbb, b7], axis=1)
    consts = {
        "onesd": ones_dr, "wo": wt.reshape(128, 4 * D),
        "cbhw": cbhw, "h5": h5, "b7t": b7t,
    }

    gs_8 = graph_sigs.astype(F8NP)
    in_maps = []
    for c in range(NCORES):
        s = slice(c * BL, (c + 1) * BL)
        rt = real[s].transpose(2, 0, 1)                      # [512, BL, 30]
        rtp = np.zeros((F_IN, BL, NP), dtype=F16NP)
        rtp[:, :, :N] = rt
        gsc = np.ascontiguousarray(
            gs_8[s]
            .reshape(8, 2, 2, 2, T, NN)
            .transpose(0, 3, 4, 2, 1, 5)
            .reshape(8, 128, 2, 2, NN)
        )
        in_maps.append(
            {
                "gs": gsc,
                "rtp": np.ascontiguousarray(
                    rtp.reshape(F_IN, 8, 256).transpose(1, 0, 2)
                ),
                **consts,
            }
        )
    return in_maps, k


def kernel(real, imag, graph_sigs, W, conv_bias, pool_w, pool_b, head_w, head_b):
    del imag  # unused by the forward pass
    in_maps, k = _prepare(
        real, graph_sigs, W, conv_bias, pool_w, pool_b, head_w, head_b
    )
    nc = _get_nc(k)
    res = run_bass_kernel_spmd(nc, in_maps, core_ids=list(range(NCORES)))
    return np.concatenate([res.results[c]["out"] for c in range(NCORES)], axis=0)


# revision 47
# speedup vs baseline: 1.0998x; 1.0998x over previous
"""DiGCNNet forward on 8 Trainium2 NeuronCores, data-parallel over batch.

Math (per batch b):
  adj = mean_t graph_sigs[b]                  # [30, 30]
  xw  = real[b] @ W                           # [30, 256]
  agg = adj^T @ xw + conv_bias                # [30, 256]
  h   = relu(agg)
  ns  = h @ pool_w + pool_b                   # [30]
  lg  = ns @ head_w^T + head_b                # [7]
  out = softmax(lg)

Final design (64 batches/core, 16 groups of 4, 4 quarters of 16):
  - gs shipped fp8e4m3 (quarter DMA traffic); T-reduce as fp8 DoubleRow
    matmuls (K=256: 4 batches per mm) accumulating a quarter into a PSUM
    tile [16, 900]; rel err ~8e-3 (tolerance 2e-2).
  - real shipped fp16, padded to 32 cols/batch, as 4 independent tiles so
    early groups don't wait on the whole tensor.
  - adjacency: one [16,900] PSUM->SBUF copy per quarter (ACT/DVE alternate)
    into padded [16, 960] staging rows (pad slots pre-set to 1.0), then ONE
    gpsimd reshape DMA per group -> ablk[128, g, 32] (the 1.0 pads become
    the K=31 conv_bias ones-rows).
  - agg: per-batch fp16 matmuls on 32-aligned PE quadrants; conv_bias rows
    injected into xwb by the PSUM->SBUF tensor_add with cbfull; the xw of
    group g+1 is emitted before the agg of group g (software pipelining).
  - pool: W pre-scaled by |pool_w|, columns permuted positives-first; ACT
    relu+accum gives nsP, DVE max+accum gives nsN; head matmuls (batched at
    the end) compute nsP@hw - nsN@hw via rhs [128, 2] -> out [28, 2].
  - head bias folded into the exp() bias AP; softmax tail on [28, 16] once.
"""

from contextlib import ExitStack

import numpy as np
import ml_dtypes

import concourse.bacc as bacc
import concourse.bass as bass
import concourse.tile as tile
from concourse import mybir
from concourse.bass_utils import run_bass_kernel_spmd

F32 = mybir.dt.float32
F16 = mybir.dt.float16
F8 = mybir.dt.float8e4
F16NP = np.float16
F8NP = ml_dtypes.float8_e4m3

B, T, N = 512, 64, 30
F_IN, D, C = 512, 256, 7
NCORES = 8
BL = B // NCORES        # 64 batches per core
NN = N * N              # 900
NG = 16                 # groups of 4 batches
NP = 32                 # padded per-batch stride (partitions / realt cols)


def _build_nc(k):
    """k = number of (permuted-first) non-negative pool_w columns."""
    assert 1 <= k <= D - 1
    nc = bacc.Bacc(None, target_bir_lowering=False)

    # gs pre-tiled fp8: [8 loads, 128=(b&1,t) part, (h, u-pair, i*30+j)]
    gs = nc.dram_tensor("gs", (8, 128, 2, 2, NN), F8, kind="ExternalInput")
    rtp = nc.dram_tensor("rtp", (4, F_IN, 512), F16, kind="ExternalInput")
    onesd = nc.dram_tensor("onesd", (128, 128), F8, kind="ExternalInput")
    wo = nc.dram_tensor("wo", (128, 4 * D), F16, kind="ExternalInput")
    cbhw = nc.dram_tensor("cbhw", (128, D + 4 * C), F32, kind="ExternalInput")
    h5 = nc.dram_tensor("h5", (4 * C, 5), F32, kind="ExternalInput")
    b7t = nc.dram_tensor("b7t", (4, 4 * C), F32, kind="ExternalInput")
    out = nc.dram_tensor("out", (BL, C), F32, kind="ExternalOutput")

    with tile.TileContext(nc) as tc, ExitStack() as ctx:
        consts = ctx.enter_context(tc.tile_pool(name="consts", bufs=1))
        gt_pool = ctx.enter_context(tc.tile_pool(name="gt", bufs=8))
        xwb_pool = ctx.enter_context(tc.tile_pool(name="xwb", bufs=3))
        h_pool = ctx.enter_context(tc.tile_pool(name="h", bufs=2))
        tail_pool = ctx.enter_context(tc.tile_pool(name="tail", bufs=1))
        adjp_pool = ctx.enter_context(
            tc.tile_pool(name="adjp", bufs=1, space=bass.MemorySpace.PSUM)
        )
        xwp_pool = ctx.enter_context(
            tc.tile_pool(name="xwp", bufs=2, space=bass.MemorySpace.PSUM)
        )
        aggp_pool = ctx.enter_context(
            tc.tile_pool(name="aggp", bufs=2, space=bass.MemorySpace.PSUM)
        )
        logp_pool = ctx.enter_context(
            tc.tile_pool(name="logp", bufs=1, space=bass.MemorySpace.PSUM)
        )

        def load_const(dram, shape, dtype):
            t = consts.tile(shape, dtype, tag=dram.name)
            nc.scalar.dma_start(t[:], dram[:])
            return t

        # tiny selectors first (the first T-reduce blocks on them)
        onesd_sb = load_const(onesd, [128, 128], F8)
        wo_sb = load_const(wo, [128, 4 * D], F16)
        cbhw_sb = load_const(cbhw, [128, D + 4 * C], F32)
        h5_sb = load_const(h5, [4 * C, 5], F32)
        b7t_sb = load_const(b7t, [4, 4 * C], F32)
        wt_sb = wo_sb[:, 0 : 4 * D].rearrange("p (c d) -> p c d", c=4)
        ones_dr = onesd_sb[:].rearrange("p (u h m) -> p u h m", u=4, h=2)
        cb_sb = cbhw_sb[:, 0:D]
        hw_sb = cbhw_sb[:, D : D + 4 * C]
        hbb_sb = h5_sb[:, 0:1]
        b7_sb = h5_sb[:, 1:5]

        rtq = []
        for rc in range(4):
            t = consts.tile([128, 4, 512], F16, tag=f"rtq{rc}")
            nc.scalar.dma_start(t[:], rtp[rc].rearrange("(c p) m -> p c m", p=128))
            rtq.append(t)

        # block-adjacency store: [128, 16 groups, 32]; all-ones memset gives
        # finite pad cols (30,31) for the M=32 agg matmuls
        ablk = consts.tile([128, NG, NP], F16, tag="ablk")
        nc.vector.memset(ablk[:], 1.0)

        # adjacency staging rows, padded to 32 elem-slots per i so the
        # per-group reshape DMA is a clean 2D->2D partition split; the pad
        # slots (1.0) become the K=31 conv_bias ones-rows in ablk
        adjs_a = consts.tile([16, NP * N], F16, tag="adjs_a")
        adjs_b = consts.tile([16, NP * N], F16, tag="adjs_b")
        nc.vector.memset(adjs_a[:, NN : NP * N], 1.0)
        nc.vector.memset(adjs_b[:, NN : NP * N], 1.0)

        ns_all = consts.tile([128, NG, 2], F32, tag="ns_all")
        logp_t = logp_pool.tile([4 * C, 2 * NG], F32, tag="logits")

        gtiles = []
        for v in range(8):
            gt = gt_pool.tile([128, 2, 2, NN], F8, tag="gt")
            nc.sync.dma_start(gt[:], gs[v])
            gtiles.append(gt)

        def emit_tred(q, u2, adjp_t):
            gt = gtiles[2 * q + u2 // 2]
            uu = u2 % 2
            for c0, c1 in ((0, 512), (512, NN)):
                nc.tensor.matmul(
                    adjp_t[:, c0:c1], ones_dr[:, u2, :, :], gt[:, :, uu, c0:c1],
                    start=(u2 == 0), stop=(u2 == 3),
                    perf_mode=mybir.MatmulPerfMode.DoubleRow,
                )

        def emit_adj_finish(q, adjp_t):
            adjs_t = adjs_a if q % 2 == 0 else adjs_b
            if q % 2 == 0:
                nc.scalar.copy(adjs_t[:, 0:NN], adjp_t[:])
            else:
                nc.vector.tensor_copy(adjs_t[:, 0:NN], adjp_t[:])
            for g2 in range(4):
                g = 4 * q + g2
                nc.gpsimd.dma_start(
                    ablk[:, g, 0:N], adjs_t[4 * g2 : 4 * g2 + 4, :]
                )

        xwbs = {}

        def emit_xw(g):
            xwp_t = xwp_pool.tile([128, D], F32, tag="xwp")
            for c4 in range(4):
                nc.tensor.matmul(
                    xwp_t[:],
                    rtq[g // 4][:, c4, 128 * (g % 4) : 128 * (g % 4 + 1)],
                    wt_sb[:, c4, :],
                    start=(c4 == 0), stop=(c4 == 3),
                )
            xwb_t = xwb_pool.tile([128, D], F16, tag="xwb")
            nc.vector.tensor_add(xwb_t[:], xwp_t[:], cb_sb[:])
            xwbs[g] = xwb_t

        def emit_rest(g):
            xwb_t = xwbs.pop(g)
            aggp_t = aggp_pool.tile([128, D], F32, tag="aggp")
            for b in range(4):
                p0 = NP * b
                nc.tensor.matmul(
                    aggp_t[p0 : p0 + NP, :],
                    ablk[p0 : p0 + 31, g, :],
                    xwb_t[p0 : p0 + 31, :],
                    start=True, stop=True, tile_position=(p0, p0),
                )
            h_t = h_pool.tile([128, D], F32, tag="h")
            nc.scalar.activation(
                h_t[:, 0:k], aggp_t[:, 0:k],
                mybir.ActivationFunctionType.Relu, accum_out=ns_all[:, g, 0:1],
            )
            nc.vector.tensor_scalar(
                h_t[:, k:D], aggp_t[:, k:D], 0.0, 0.0,
                mybir.AluOpType.max, mybir.AluOpType.add,
                accum_out=ns_all[:, g, 1:2],
            )

        # ---- pipelined emission: 4 quarters of 16 batches; the xw of
        # group g+1 is emitted before the agg of group g so the PE keeps
        # streaming through the xwb (DVE) latency
        for q in range(4):
            adjp_t = adjp_pool.tile([16, NN], F32, tag="adjp")
            for u2 in range(4):
                emit_tred(q, u2, adjp_t)
                if q >= 1:
                    g = 4 * (q - 1) + u2
                    emit_xw(g)
                    if g >= 1:
                        emit_rest(g - 1)
            emit_adj_finish(q, adjp_t)
        for g2 in range(4):
            emit_xw(12 + g2)
            emit_rest(11 + g2)
        emit_rest(15)

        # ---- head matmuls batched: lg[28, 2] per group ----
        for g in range(NG):
            nc.tensor.matmul(
                logp_t[:, 2 * g : 2 * g + 2], hw_sb, ns_all[:, g, :],
                start=True, stop=True,
            )

        # ---- softmax tail over the 7-class blocks ----
        lgs_t = tail_pool.tile([4 * C, 2 * NG], F32, tag="lgs")
        nc.vector.tensor_copy(lgs_t[:], logp_t[:])
        lgd_t = tail_pool.tile([4 * C, NG], F32, tag="lgd")
        nc.vector.tensor_sub(
            lgd_t[:],
            lgs_t[:].rearrange("p (g two) -> p two g", two=2)[:, 0, :],
            lgs_t[:].rearrange("p (g two) -> p two g", two=2)[:, 1, :],
        )
        e_t = tail_pool.tile([4 * C, NG], F32, tag="e")
        nc.scalar.activation(
            e_t[:], lgd_t[:], mybir.ActivationFunctionType.Exp, bias=hbb_sb,
        )
        # tail matmuls reuse sub-regions of the (already consumed) logits bank
        sum_p = logp_t[0:4, 0:NG]
        nc.tensor.matmul(sum_p, b7_sb, e_t[:], start=True, stop=True)
        ssb_t = tail_pool.tile([4, NG], F32, tag="ssb")
        nc.vector.tensor_copy(ssb_t[:], sum_p)
        bc_p = logp_t[:, NG : 2 * NG]
        nc.tensor.matmul(bc_p, b7t_sb[:], ssb_t[:], start=True, stop=True)
        rs_t = tail_pool.tile([4 * C, NG], F32, tag="rs")
        nc.vector.reciprocal(rs_t[:], bc_p)
        res_t = tail_pool.tile([4 * C, NG], F32, tag="res")
        nc.vector.tensor_mul(res_t[:], e_t[:], rs_t[:])
        nc.sync.dma_start(out.rearrange("(g bi) c -> (bi c) g", bi=4), res_t[:])

    nc.compile()
    return nc


_NC_CACHE = {}


def _get_nc(k):
    if k not in _NC_CACHE:
        _NC_CACHE[k] = _build_nc(k)
    return _NC_CACHE[k]


def _f32(x):
    return np.asarray(x, dtype=np.float32)


def _prepare(real, graph_sigs, W, conv_bias, pool_w, pool_b, head_w, head_b):
    real = _f32(real)
    graph_sigs = _f32(graph_sigs)
    W = _f32(W)
    conv_bias = _f32(conv_bias)
    pool_w = _f32(pool_w)
    head_w = _f32(head_w)
    head_b = _f32(head_b)

    # permute feature columns: non-negative pool_w first; fold |pool_w| into W
    nonneg = pool_w >= 0
    perm = np.argsort(~nonneg, kind="stable")
    k = int(nonneg.sum())
    apw = np.abs(pool_w)[perm]
    Wp = np.ascontiguousarray((W[:, perm] * apw[None, :]).astype(F16NP))
    cbp = (conv_bias[perm] * apw).astype(np.float32)

    wt = np.ascontiguousarray(Wp.reshape(4, 128, D).transpose(1, 0, 2))

    # DoubleRow selectors: m = 4*u + 2*h + (p//64), u = within-load half
    ones_dr = np.zeros((2, 64, 4, 2, 16), dtype=F8NP)
    for c in range(2):
        for u in range(4):
            for h in range(2):
                ones_dr[c, :, u, h, 4 * u + 2 * h + c] = F8NP(1.0 / T)
    ones_dr = ones_dr.reshape(128, 128)

    cbfull = np.zeros((128, D), dtype=np.float32)
    for b in range(4):
        cbfull[NP * b + N, :] = cbp

    hwblk = np.zeros((128, 4 * C), dtype=np.float32)
    for b in range(4):
        hwblk[NP * b : NP * b + N, C * b : C * (b + 1)] = head_w.T
    hb_eff = head_b + np.float32(np.asarray(pool_b)) * head_w.sum(axis=1)
    hbb = np.tile(hb_eff, 4).reshape(4 * C, 1).astype(np.float32)

    b7 = np.zeros((4 * C, 4), dtype=np.float32)
    for b in range(4):
        b7[C * b : C * (b + 1), b] = 1.0
    b7t = np.ascontiguousarray(b7.T)

    cbhw = np.concatenate([cbfull, hwblk], axis=1)
    h5 = np.concatenate([hbb, b7], axis=1)
    consts = {
        "onesd": ones_dr, "wo": wt.reshape(128, 4 * D),
        "cbhw": cbhw, "h5": h5, "b7t": b7t,
    }

    gs_8 = graph_sigs.astype(F8NP)
    in_maps = []
    for c in range(NCORES):
        s = slice(c * BL, (c + 1) * BL)
        rt = real[s].transpose(2, 0, 1)                      # [512, BL, 30]
        rtp = np.zeros((F_IN, BL, NP), dtype=F16NP)
        rtp[:, :, :N] = rt
        gsc = np.ascontiguousarray(
            gs_8[s]
            .reshape(8, 2, 2, 2, T, NN)
            .transpose(0, 3, 4, 2, 1, 5)
            .reshape(8, 128, 2, 2, NN)
        )
        in_maps.append(
            {
                "gs": gsc,
                "rtp": np.ascontiguousarray(
                    rtp.reshape(F_IN, 4, 512).transpose(1, 0, 2)
                ),
                **consts,
            }
        )
    return in_maps, k


def kernel(real, imag, graph_sigs, W, conv_bias, pool_w, pool_b, head_w, head_b):
    del imag  # unused by the forward pass
    in_maps, k = _prepare(
        real, graph_sigs, W, conv_bias, pool_w, pool_b, head_w, head_b
    )
    nc = _get_nc(k)
    res = run_bass_kernel_spmd(nc, in_maps, core_ids=list(range(NCORES)))
    return np.concatenate([res.results[c]["out"] for c in range(NCORES)], axis=0)


# revision 48
# speedup vs baseline: 1.1321x; 1.0294x over previous
"""DiGCNNet forward on 8 Trainium2 NeuronCores, data-parallel over batch.

Math (per batch b):
  adj = mean_t graph_sigs[b]                  # [30, 30]
  xw  = real[b] @ W                           # [30, 256]
  agg = adj^T @ xw + conv_bias                # [30, 256]
  h   = relu(agg)
  ns  = h @ pool_w + pool_b                   # [30]
  lg  = ns @ head_w^T + head_b                # [7]
  out = softmax(lg)

Final design (64 batches/core, 16 groups of 4, 4 quarters of 16):
  - gs shipped fp8e4m3 (quarter DMA traffic); T-reduce as fp8 DoubleRow
    matmuls (K=256: 4 batches per mm) accumulating a quarter into a PSUM
    tile [16, 900]; rel err ~8e-3 (tolerance 2e-2).
  - real shipped fp16, padded to 32 cols/batch, as 4 independent tiles so
    early groups don't wait on the whole tensor.
  - adjacency: one [16,900] PSUM->SBUF copy per quarter (ACT/DVE alternate)
    into padded [16, 960] staging rows (pad slots pre-set to 1.0), then ONE
    gpsimd reshape DMA per group -> ablk[128, g, 32] (the 1.0 pads become
    the K=31 conv_bias ones-rows).
  - agg: per-batch fp16 matmuls on 32-aligned PE quadrants; conv_bias rows
    injected into xwb by the PSUM->SBUF tensor_add with cbfull; the xw of
    group g+1 is emitted before the agg of group g (software pipelining).
  - pool: W pre-scaled by |pool_w|, columns permuted positives-first; ACT
    relu+accum gives nsP, DVE max+accum gives nsN; head matmuls (batched at
    the end) compute nsP@hw - nsN@hw via rhs [128, 2] -> out [28, 2].
  - head bias folded into the exp() bias AP; softmax tail on [28, 16] once.
"""

from contextlib import ExitStack

import numpy as np
import ml_dtypes

import concourse.bacc as bacc
import concourse.bass as bass
import concourse.tile as tile
from concourse import mybir
from concourse.bass_utils import run_bass_kernel_spmd

F32 = mybir.dt.float32
F16 = mybir.dt.float16
F8 = mybir.dt.float8e4
F16NP = np.float16
F8NP = ml_dtypes.float8_e4m3

B, T, N = 512, 64, 30
F_IN, D, C = 512, 256, 7
NCORES = 8
BL = B // NCORES        # 64 batches per core
NN = N * N              # 900
NG = 16                 # groups of 4 batches
NP = 32                 # padded per-batch stride (partitions / realt cols)


def _build_nc(k):
    """k = number of (permuted-first) non-negative pool_w columns."""
    assert 1 <= k <= D - 1
    nc = bacc.Bacc(None, target_bir_lowering=False)

    # gs pre-tiled fp8: [8 loads, 128=(b&1,t) part, (h, u-pair, i*30+j)]
    gs = nc.dram_tensor("gs", (8, 128, 2, 2, NN), F8, kind="ExternalInput")
    rtp = nc.dram_tensor("rtp", (4, F_IN, 512), F16, kind="ExternalInput")
    onesd = nc.dram_tensor("onesd", (128, 128), F8, kind="ExternalInput")
    wo = nc.dram_tensor("wo", (128, 4 * D), F16, kind="ExternalInput")
    cbhw = nc.dram_tensor("cbhw", (128, D + 4 * C), F32, kind="ExternalInput")
    h5 = nc.dram_tensor("h5", (4 * C, 5), F32, kind="ExternalInput")
    b7t = nc.dram_tensor("b7t", (4, 4 * C), F32, kind="ExternalInput")
    out = nc.dram_tensor("out", (BL, C), F32, kind="ExternalOutput")

    with tile.TileContext(nc) as tc, ExitStack() as ctx:
        consts = ctx.enter_context(tc.tile_pool(name="consts", bufs=1))
        gt_pool = ctx.enter_context(tc.tile_pool(name="gt", bufs=8))
        xwb_pool = ctx.enter_context(tc.tile_pool(name="xwb", bufs=4))
        h_pool = ctx.enter_context(tc.tile_pool(name="h", bufs=2))
        tail_pool = ctx.enter_context(tc.tile_pool(name="tail", bufs=1))
        adjp_pool = ctx.enter_context(
            tc.tile_pool(name="adjp", bufs=1, space=bass.MemorySpace.PSUM)
        )
        xwp_pool = ctx.enter_context(
            tc.tile_pool(name="xwp", bufs=2, space=bass.MemorySpace.PSUM)
        )
        aggp_pool = ctx.enter_context(
            tc.tile_pool(name="aggp", bufs=3, space=bass.MemorySpace.PSUM)
        )
        logp_pool = ctx.enter_context(
            tc.tile_pool(name="logp", bufs=1, space=bass.MemorySpace.PSUM)
        )

        def load_const(dram, shape, dtype):
            t = consts.tile(shape, dtype, tag=dram.name)
            nc.scalar.dma_start(t[:], dram[:])
            return t

        # tiny selectors first (the first T-reduce blocks on them)
        onesd_sb = load_const(onesd, [128, 128], F8)
        wo_sb = load_const(wo, [128, 4 * D], F16)
        cbhw_sb = load_const(cbhw, [128, D + 4 * C], F32)
        h5_sb = load_const(h5, [4 * C, 5], F32)
        b7t_sb = load_const(b7t, [4, 4 * C], F32)
        wt_sb = wo_sb[:, 0 : 4 * D].rearrange("p (c d) -> p c d", c=4)
        ones_dr = onesd_sb[:].rearrange("p (u h m) -> p u h m", u=4, h=2)
        cb_sb = cbhw_sb[:, 0:D]
        hw_sb = cbhw_sb[:, D : D + 4 * C]
        hbb_sb = h5_sb[:, 0:1]
        b7_sb = h5_sb[:, 1:5]

        rtq = []
        for rc in range(4):
            t = consts.tile([128, 4, 512], F16, tag=f"rtq{rc}")
            nc.scalar.dma_start(t[:], rtp[rc].rearrange("(c p) m -> p c m", p=128))
            rtq.append(t)

        # block-adjacency store: [128, 16 groups, 32]; all-ones memset gives
        # finite pad cols (30,31) for the M=32 agg matmuls
        ablk = consts.tile([128, NG, NP], F16, tag="ablk")
        nc.vector.memset(ablk[:], 1.0)

        # adjacency staging rows, padded to 32 elem-slots per i so the
        # per-group reshape DMA is a clean 2D->2D partition split; the pad
        # slots (1.0) become the K=31 conv_bias ones-rows in ablk
        adjs_a = consts.tile([16, NP * N], F16, tag="adjs_a")
        adjs_b = consts.tile([16, NP * N], F16, tag="adjs_b")
        nc.vector.memset(adjs_a[:, NN : NP * N], 1.0)
        nc.vector.memset(adjs_b[:, NN : NP * N], 1.0)

        ns_all = consts.tile([128, NG, 2], F32, tag="ns_all")
        logp_t = logp_pool.tile([4 * C, 2 * NG], F32, tag="logits")

        gtiles = []
        for v in range(8):
            gt = gt_pool.tile([128, 2, 2, NN], F8, tag="gt")
            nc.sync.dma_start(gt[:], gs[v])
            gtiles.append(gt)

        def emit_tred(q, u2, adjp_t):
            gt = gtiles[2 * q + u2 // 2]
            uu = u2 % 2
            for c0, c1 in ((0, 512), (512, NN)):
                nc.tensor.matmul(
                    adjp_t[:, c0:c1], ones_dr[:, u2, :, :], gt[:, :, uu, c0:c1],
                    start=(u2 == 0), stop=(u2 == 3),
                    perf_mode=mybir.MatmulPerfMode.DoubleRow,
                )

        def emit_adj_finish(q, adjp_t):
            adjs_t = adjs_a if q % 2 == 0 else adjs_b
            if q % 2 == 0:
                nc.scalar.copy(adjs_t[:, 0:NN], adjp_t[:])
            else:
                nc.vector.tensor_copy(adjs_t[:, 0:NN], adjp_t[:])
            for g2 in range(4):
                g = 4 * q + g2
                eng = nc.gpsimd if g2 % 2 == 0 else nc.scalar
                eng.dma_start(ablk[:, g, 0:N], adjs_t[4 * g2 : 4 * g2 + 4, :])

        xwbs = {}

        def emit_xw(g):
            xwp_t = xwp_pool.tile([128, D], F32, tag="xwp")
            for c4 in range(4):
                nc.tensor.matmul(
                    xwp_t[:],
                    rtq[g // 4][:, c4, 128 * (g % 4) : 128 * (g % 4 + 1)],
                    wt_sb[:, c4, :],
                    start=(c4 == 0), stop=(c4 == 3),
                )
            xwb_t = xwb_pool.tile([128, D], F16, tag="xwb")
            nc.vector.tensor_add(xwb_t[:], xwp_t[:], cb_sb[:])
            xwbs[g] = xwb_t

        def emit_rest(g):
            xwb_t = xwbs.pop(g)
            aggp_t = aggp_pool.tile([128, D], F32, tag="aggp")
            for b in range(4):
                p0 = NP * b
                nc.tensor.matmul(
                    aggp_t[p0 : p0 + NP, :],
                    ablk[p0 : p0 + 31, g, :],
                    xwb_t[p0 : p0 + 31, :],
                    start=True, stop=True, tile_position=(p0, p0),
                )
            h_t = h_pool.tile([128, D], F32, tag="h")
            nc.scalar.activation(
                h_t[:, 0:k], aggp_t[:, 0:k],
                mybir.ActivationFunctionType.Relu, accum_out=ns_all[:, g, 0:1],
            )
            nc.vector.tensor_scalar(
                h_t[:, k:D], aggp_t[:, k:D], 0.0, 0.0,
                mybir.AluOpType.max, mybir.AluOpType.add,
                accum_out=ns_all[:, g, 1:2],
            )

        # ---- pipelined emission: 4 quarters of 16 batches; the xw of
        # group g+1 is emitted before the agg of group g so the PE keeps
        # streaming through the xwb (DVE) latency
        for q in range(4):
            adjp_t = adjp_pool.tile([16, NN], F32, tag="adjp")
            for u2 in range(4):
                emit_tred(q, u2, adjp_t)
                if q >= 1:
                    g = 4 * (q - 1) + u2
                    emit_xw(g)
                    if g >= 2:
                        emit_rest(g - 2)
            emit_adj_finish(q, adjp_t)
        for g2 in range(4):
            emit_xw(12 + g2)
            emit_rest(10 + g2)
        emit_rest(14)
        emit_rest(15)

        # ---- head matmuls batched: lg[28, 2] per group ----
        for g in range(NG):
            nc.tensor.matmul(
                logp_t[:, 2 * g : 2 * g + 2], hw_sb, ns_all[:, g, :],
                start=True, stop=True,
            )

        # ---- softmax tail over the 7-class blocks ----
        lgs_t = tail_pool.tile([4 * C, 2 * NG], F32, tag="lgs")
        nc.vector.tensor_copy(lgs_t[:], logp_t[:])
        lgd_t = tail_pool.tile([4 * C, NG], F32, tag="lgd")
        nc.vector.tensor_sub(
            lgd_t[:],
            lgs_t[:].rearrange("p (g two) -> p two g", two=2)[:, 0, :],
            lgs_t[:].rearrange("p (g two) -> p two g", two=2)[:, 1, :],
        )
        e_t = tail_pool.tile([4 * C, NG], F32, tag="e")
        nc.scalar.activation(
            e_t[:], lgd_t[:], mybir.ActivationFunctionType.Exp, bias=hbb_sb,
        )
        # tail matmuls reuse sub-regions of the (already consumed) logits bank
        sum_p = logp_t[0:4, 0:NG]
        nc.tensor.matmul(sum_p, b7_sb, e_t[:], start=True, stop=True)
        ssb_t = tail_pool.tile([4, NG], F32, tag="ssb")
        nc.vector.tensor_copy(ssb_t[:], sum_p)
        bc_p = logp_t[:, NG : 2 * NG]
        nc.tensor.matmul(bc_p, b7t_sb[:], ssb_t[:], start=True, stop=True)
        rs_t = tail_pool.tile([4 * C, NG], F32, tag="rs")
        nc.vector.reciprocal(rs_t[:], bc_p)
        res_t = tail_pool.tile([4 * C, NG], F32, tag="res")
        nc.vector.tensor_mul(res_t[:], e_t[:], rs_t[:])
        nc.sync.dma_start(out.rearrange("(g bi) c -> (bi c) g", bi=4), res_t[:])

    nc.compile()
    return nc


_NC_CACHE = {}


def _get_nc(k):
    if k not in _NC_CACHE:
        _NC_CACHE[k] = _build_nc(k)
    return _NC_CACHE[k]


def _f32(x):
    return np.asarray(x, dtype=np.float32)


def _prepare(real, graph_sigs, W, conv_bias, pool_w, pool_b, head_w, head_b):
    real = _f32(real)
    graph_sigs = _f32(graph_sigs)
    W = _f32(W)
    conv_bias = _f32(conv_bias)
    pool_w = _f32(pool_w)
    head_w = _f32(head_w)
    head_b = _f32(head_b)

    # permute feature columns: non-negative pool_w first; fold |pool_w| into W
    nonneg = pool_w >= 0
    perm = np.argsort(~nonneg, kind="stable")
    k = int(nonneg.sum())
    apw = np.abs(pool_w)[perm]
    Wp = np.ascontiguousarray((W[:, perm] * apw[None, :]).astype(F16NP))
    cbp = (conv_bias[perm] * apw).astype(np.float32)

    wt = np.ascontiguousarray(Wp.reshape(4, 128, D).transpose(1, 0, 2))

    # DoubleRow selectors: m = 4*u + 2*h + (p//64), u = within-load half
    ones_dr = np.zeros((2, 64, 4, 2, 16), dtype=F8NP)
    for c in range(2):
        for u in range(4):
            for h in range(2):
                ones_dr[c, :, u, h, 4 * u + 2 * h + c] = F8NP(1.0 / T)
    ones_dr = ones_dr.reshape(128, 128)

    cbfull = np.zeros((128, D), dtype=np.float32)
    for b in range(4):
        cbfull[NP * b + N, :] = cbp

    hwblk = np.zeros((128, 4 * C), dtype=np.float32)
    for b in range(4):
        hwblk[NP * b : NP * b + N, C * b : C * (b + 1)] = head_w.T
    hb_eff = head_b + np.float32(np.asarray(pool_b)) * head_w.sum(axis=1)
    hbb = np.tile(hb_eff, 4).reshape(4 * C, 1).astype(np.float32)

    b7 = np.zeros((4 * C, 4), dtype=np.float32)
    for b in range(4):
        b7[C * b : C * (b + 1), b] = 1.0
    b7t = np.ascontiguousarray(b7.T)

    cbhw = np.concatenate([cbfull, hwblk], axis=1)
    h5 = np.concatenate([hbb, b7], axis=1)
    consts = {
        "onesd": ones_dr, "wo": wt.reshape(128, 4 * D),
        "cbhw": cbhw, "h5": h5, "b7t": b7t,
    }

    gs_8 = graph_sigs.astype(F8NP)
    in_maps = []
    for c in range(NCORES):
        s = slice(c * BL, (c + 1) * BL)
        rt = real[s].transpose(2, 0, 1)                      # [512, BL, 30]
        rtp = np.zeros((F_IN, BL, NP), dtype=F16NP)
        rtp[:, :, :N] = rt
        gsc = np.ascontiguousarray(
            gs_8[s]
            .reshape(8, 2, 2, 2, T, NN)
            .transpose(0, 3, 4, 2, 1, 5)
            .reshape(8, 128, 2, 2, NN)
        )
        in_maps.append(
            {
                "gs": gsc,
                "rtp": np.ascontiguousarray(
                    rtp.reshape(F_IN, 4, 512).transpose(1, 0, 2)
                ),
                **consts,
            }
        )
    return in_maps, k


def kernel(real, imag, graph_sigs, W, conv_bias, pool_w, pool_b, head_w, head_b):
    del imag  # unused by the forward pass
    in_maps, k = _prepare(
        real, graph_sigs, W, conv_bias, pool_w, pool_b, head_w, head_b
    )
    nc = _get_nc(k)
    res = run_bass_kernel_spmd(nc, in_maps, core_ids=list(range(NCORES)))
    return np.concatenate([res.results[c]["out"] for c in range(NCORES)], axis=0)
